# revision 4
# baseline (speedup 1.0000x reference)
"""Trainium2 Bass kernel for nn_DigitConvolutionalModel.

Model: x(B,784) -> reshape 28x28 -> 3x3 valid cross-correlation (kernel is an
input) -> flatten 676 -> Linear(676,128)+ReLU -> Linear(128,10).

Strategy:
  * Fold the 3x3 conv into the first linear layer on the host: the conv is a
    linear map, so h = relu(x @ W1eff.T + b1) with W1eff (128, 784) built by
    scattering conv_w-weighted copies of w1 onto the 28x28 grid. The device
    kernel is then a plain 2-layer MLP over 784 features.
  * Pure data parallelism: batch 65536 split as 8192 rows per NeuronCore,
    weights replicated.
  * Activations are shipped feature-major and fp16 (PE runs fp16 at full
    rate; the per-core HBM ceiling is the roofline, so halving bytes halves
    the kernel time; measured error ~5e-4 of scale). The kernel computes
    logits^T = w2 @ relu(W1eff @ x^T + b1) + b2 and the host transposes the
    gathered (10, B) result back.
  * x is shipped packed per DMA block with each partition's data fully
    contiguous in HBM: a block load is 112 descriptors of 14 KB (4 KB
    descriptors leave the 16 SDMA engines descriptor-bound at ~240 GB/s).
    Block loads alternate between the two HWDGE rings (sync / scalar) so
    each SDMA engine has two descriptors in flight to hide HBM latency.
  * Weights load first so they land before block 0; layer-2 work for block
    b is emitted between block b+1's layer-1 matmuls so the PE FIFO never
    waits on the DVE epilogue; per-block (not per-tile) epilogue ops.
"""

from contextlib import ExitStack

import numpy as np

B = 65536
H = W = 28
K = 3
CH = CW = 26
FEAT = H * W          # 784
HID = 128
OUT = 10
NCORES = 8
BC = B // NCORES      # 8192 rows per core

KC = 112              # contraction-chunk partition size
KCH = 7               # chunks: 7 * 112 = 784
NT = 512              # batch rows per matmul (one PSUM bank fp32)
XB = 1024             # batch rows per DMA block
WCOL = KCH * HID      # 896 w1t columns in the packed weight tile

VARIANT = "f16"

_NC_CACHE = {}


def _blocks(bc):
    return [min(XB, bc - o) for o in range(0, bc, XB)]


def _dtypes(variant):
    import concourse.mybir as mybir

    f32 = mybir.dt.float32
    if variant == "f32":
        return f32, f32
    if variant == "bf16":
        return mybir.dt.bfloat16, mybir.dt.bfloat16
    if variant == "f16":
        return mybir.dt.float16, mybir.dt.float16
    raise ValueError(variant)


def _build_nc(bc, variant):
    from concourse import bacc
    import concourse.mybir as mybir
    import concourse.tile as tile

    f32 = mybir.dt.float32
    wdt, xdt = _dtypes(variant)
    blocks = _blocks(bc)
    nblk = len(blocks)

    nc = bacc.Bacc(
        "TRN2",
        target_bir_lowering=False,
        debug=False,
        enable_asserts=False,
        num_devices=NCORES,
    )
    # [112, 7*bc] with per-block column groups: block b at columns
    # [7*off_b, 7*(off_b+xb)), laid out [chunk, row] so each partition's
    # block data is one contiguous run -> one big DMA descriptor per
    # partition per block
    xT = nc.dram_tensor("xT", [KC, KCH * bc], xdt, kind="ExternalInput").ap()
    # wpk: cols 0..895 = w1t as [112, 7, 128] (rows 112-127 zero),
    #      cols 896..905 = w2t [128, 10]
    wpk = nc.dram_tensor("wpk", [HID, WCOL + OUT], wdt, kind="ExternalInput").ap()
    # bpk: col 0 = b1 [128], col 1 rows 0..9 = b2
    bpk = nc.dram_tensor("bpk", [HID, 2], f32, kind="ExternalInput").ap()
    outT = nc.dram_tensor("outT", [OUT, bc], f32, kind="ExternalOutput").ap()

    with ExitStack() as ctx:
        tc = ctx.enter_context(tile.TileContext(nc))
        wpool = ctx.enter_context(tc.tile_pool(name="w", bufs=1))
        xpool = ctx.enter_context(tc.tile_pool(name="x", bufs=nblk))
        hpool = ctx.enter_context(tc.tile_pool(name="h", bufs=2))
        opool = ctx.enter_context(tc.tile_pool(name="o", bufs=3))
        p1pool = ctx.enter_context(tc.tile_pool(name="p1", bufs=2, space="PSUM"))
        p2pool = ctx.enter_context(tc.tile_pool(name="p2", bufs=2, space="PSUM"))

        # weights first: they must land before block 0 so the first
        # matmul isn't starved (engines round-robin rings per packet, so
        # anything queued after the bulk x stream finishes ~2 blocks in)
        ws = wpool.tile([HID, WCOL + OUT], wdt)
        bs = wpool.tile([HID, 2], f32)
        nc.scalar.dma_start(ws[:], wpk[:])
        nc.scalar.dma_start(bs[:], bpk[:])
        b1s = bs[:, 0:1]
        b2s = bs[0:OUT, 1:2]
        w2s = ws[:, WCOL : WCOL + OUT]

        # all block loads issued up front (bufs=nblk: no WAR stalls in the
        # ring FIFOs), alternating rings
        xs_list = []
        off = 0
        for blk, xb in enumerate(blocks):
            xs = xpool.tile([KC, KCH * xb], xdt, tag="xs", name=f"xs_{blk}")
            eng = nc.sync if blk % 2 == 0 else nc.scalar
            eng.dma_start(xs[:], xT[:, KCH * off : KCH * (off + xb)])
            xs_list.append(xs)
            off += xb

        add = mybir.AluOpType.add
        mx = mybir.AluOpType.max

        def emit_l1(blk, xb):
            """Layer-1 matmuls for one block: accumulate into a 2-bank
            PSUM tile, 512-col half per bank."""
            xs = xs_list[blk]
            p1 = p1pool.tile([HID, xb], f32, tag="p1", name=f"p1_{blk}")
            for t0 in range(0, xb, NT):
                nt = min(NT, xb - t0)
                for c in range(KCH):
                    nc.tensor.matmul(
                        p1[:, t0 : t0 + nt],
                        ws[0:KC, c * HID : (c + 1) * HID],
                        xs[:, c * xb + t0 : c * xb + t0 + nt],
                        start=(c == 0),
                        stop=(c == KCH - 1),
                    )
            return p1

        def emit_l2(blk, xb, p1, off):
            """relu+b1 (one DVE op), layer-2 matmuls, b2-add evacuation
            (one DVE op), output store for one block."""
            hs = hpool.tile([HID, xb], xdt, tag="hs", name=f"hs_{blk}")
            nc.vector.tensor_scalar(hs[:], p1[:], b1s, 0.0, add, mx)
            p2 = p2pool.tile([OUT, xb], f32, tag="p2", name=f"p2_{blk}")
            for t0 in range(0, xb, NT):
                nt = min(NT, xb - t0)
                nc.tensor.matmul(
                    p2[:, t0 : t0 + nt],
                    w2s,
                    hs[:, t0 : t0 + nt],
                    start=True,
                    stop=True,
                )
            os_ = opool.tile([OUT, xb], f32, tag="os", name=f"os_{blk}")
            nc.vector.tensor_scalar_add(os_[:], p2[:], b2s)
            nc.scalar.dma_start(outT[:, off : off + xb], os_[:])

        offs = []
        off = 0
        for xb in blocks:
            offs.append(off)
            off += xb

        prev = None  # (blk, xb, p1, off)
        for blk, xb in enumerate(blocks):
            p1 = emit_l1(blk, xb)
            if prev is not None:
                emit_l2(*prev)
            prev = (blk, xb, p1, offs[blk])
        emit_l2(*prev)

    nc.compile()
    return nc


def get_nc(bc=BC, variant=VARIANT):
    key = (bc, variant)
    if key not in _NC_CACHE:
        _NC_CACHE[key] = _build_nc(bc, variant)
    return _NC_CACHE[key]


def _np_wdt(variant):
    if variant == "bf16":
        import ml_dtypes

        return ml_dtypes.bfloat16
    if variant == "f16":
        return np.float16
    return np.float32


def _pack_xT(shard, blocks, wnp):
    """[bc, 784] row-major shard -> [112, 7*bc] per-block-contiguous."""
    parts = []
    off = 0
    for xb in blocks:
        sub = shard[off : off + xb]  # [xb, 784]
        # [xb, 7, 112] -> [112, 7, xb] -> [112, 7*xb]
        parts.append(sub.reshape(xb, KCH, KC).transpose(2, 1, 0).reshape(KC, KCH * xb))
        off += xb
    return np.ascontiguousarray(np.concatenate(parts, axis=1)).astype(wnp, copy=False)


def _host_prep(x, conv_w, w1, b1, w2, b2, variant):
    """Fold conv into layer-1 weights and lay out per-core device inputs."""
    x = np.asarray(x, dtype=np.float32)
    conv_w = np.asarray(conv_w, dtype=np.float32)
    w1 = np.asarray(w1, dtype=np.float32)
    b1 = np.asarray(b1, dtype=np.float32)
    w2 = np.asarray(w2, dtype=np.float32)
    b2 = np.asarray(b2, dtype=np.float32)

    w1_img = w1.reshape(HID, CH, CW)
    w1eff = np.zeros((HID, H, W), dtype=np.float32)
    for di in range(K):
        for dj in range(K):
            w1eff[:, di : di + CH, dj : dj + CW] += conv_w[di, dj] * w1_img
    w1eff = w1eff.reshape(HID, FEAT)

    wnp = _np_wdt(variant)
    # w1t layout [112, 7*128]: chunk c partition p holds feature c*112+p
    w1t_host = (
        w1eff.T.reshape(KCH, KC, HID).transpose(1, 0, 2).reshape(KC, KCH * HID)
    )
    wpk_host = np.zeros((HID, WCOL + OUT), dtype=np.float32)
    wpk_host[0:KC, 0:WCOL] = w1t_host
    wpk_host[:, WCOL : WCOL + OUT] = w2.T
    wpk_host = np.ascontiguousarray(wpk_host).astype(wnp)

    bpk_host = np.zeros((HID, 2), dtype=np.float32)
    bpk_host[:, 0] = b1
    bpk_host[0:OUT, 1] = b2
    bpk_host = np.ascontiguousarray(bpk_host)

    blocks = _blocks(BC)
    xq = x.astype(wnp)
    in_maps = []
    for c in range(NCORES):
        in_maps.append(
            {
                "xT": _pack_xT(xq[c * BC : (c + 1) * BC], blocks, wnp),
                "wpk": wpk_host,
                "bpk": bpk_host,
            }
        )
    return in_maps


def run(x, conv_w, w1, b1, w2, b2, trace=False, variant=VARIANT):
    from concourse.bass_utils import run_bass_kernel_spmd

    in_maps = _host_prep(x, conv_w, w1, b1, w2, b2, variant)
    nc = get_nc(BC, variant)
    res = run_bass_kernel_spmd(nc, in_maps, list(range(NCORES)), trace=trace)
    outT = np.concatenate([r["outT"] for r in res.results], axis=1)  # [10, B]
    return np.ascontiguousarray(outT.T), res


def kernel(x, conv_w, w1, b1, w2, b2):
    out, _ = run(x, conv_w, w1, b1, w2, b2)
    return out


# revision 5
# speedup vs baseline: 1.1521x; 1.1521x over previous
"""Trainium2 Bass kernel for nn_DigitConvolutionalModel.

Model: x(B,784) -> reshape 28x28 -> 3x3 valid cross-correlation (kernel is an
input) -> flatten 676 -> Linear(676,128)+ReLU -> Linear(128,10).

Strategy:
  * Fold the 3x3 conv into the first linear layer on the host: the conv is a
    linear map, so h = relu(x @ W1eff.T + b1) with W1eff (128, 784) built by
    scattering conv_w-weighted copies of w1 onto the 28x28 grid. The device
    kernel is then a plain 2-layer MLP over 784 features.
  * Pure data parallelism: batch 65536 split as 8192 rows per NeuronCore,
    weights replicated. fp16 activations and weights (PE full rate, half
    DMA bytes; measured error ~5e-4 of scale). The kernel computes
    logits^T = w2 @ relu(W1eff @ x^T + b1) + b2; the host transposes the
    gathered (10, B) result back.
  * x is shipped packed per DMA block with each partition's block data one
    contiguous HBM run: a 2048-row block load is 112 descriptors of 28 KB.
    SDMA engines pay ~250 ns fixed per descriptor, so small descriptors
    cap throughput (4 KB -> ~240 GB/s); 28 KB descriptors reach ~21.5
    GB/s/engine. Descending block sizes keep the post-stream tail short.
  * Weights load first on the scalar ring so they land before block 0
    (engines drain rings round-robin, so anything queued after the bulk
    x stream arrives blocks late). Outputs also ride the scalar ring so a
    waiting store never head-of-line-blocks an x load on the sync ring.
  * Compute in 1024-column units; layer-2 work for unit u is emitted
    between unit u+1's layer-1 matmuls so the PE FIFO never waits on the
    DVE epilogue (relu+b1 and b2-add run as one batched DVE op per unit).
"""

from contextlib import ExitStack

import numpy as np

B = 65536
H = W = 28
K = 3
CH = CW = 26
FEAT = H * W          # 784
HID = 128
OUT = 10
NCORES = 8
BC = B // NCORES      # 8192 rows per core

KC = 112              # contraction-chunk partition size
KCH = 7               # chunks: 7 * 112 = 784
NT = 512              # batch rows per matmul (one PSUM bank fp32)
UC = 1024             # batch rows per compute unit (2 PSUM banks)
WCOL = KCH * HID      # 896 w1t columns in the packed weight tile

VARIANT = "f16"

_NC_CACHE = {}


def _blocks(bc):
    # descending: big DMAs early (descriptor efficiency), small final
    # blocks so the post-DMA compute tail is short
    if bc == 8192:
        blocks = [2048, 2048, 2048, 1024, 512, 512]
    else:
        blocks = [min(1024, bc - o) for o in range(0, bc, 1024)]
    assert sum(blocks) == bc
    return blocks


def _dtypes(variant):
    import concourse.mybir as mybir

    f32 = mybir.dt.float32
    if variant == "f32":
        return f32, f32
    if variant == "bf16":
        return mybir.dt.bfloat16, mybir.dt.bfloat16
    if variant == "f16":
        return mybir.dt.float16, mybir.dt.float16
    raise ValueError(variant)


def _build_nc(bc, variant):
    from concourse import bacc
    import concourse.mybir as mybir
    import concourse.tile as tile

    f32 = mybir.dt.float32
    wdt, xdt = _dtypes(variant)
    blocks = _blocks(bc)

    nc = bacc.Bacc(
        "TRN2",
        target_bir_lowering=False,
        debug=False,
        enable_asserts=False,
        num_devices=NCORES,
    )
    # [112, 7*bc] with per-block column groups: block b at columns
    # [7*off_b, 7*(off_b+xb)), chunk-major inside the block so each
    # partition's block data is one contiguous HBM run
    xT = nc.dram_tensor("xT", [KC, KCH * bc], xdt, kind="ExternalInput").ap()
    # wpk: cols 0..895 = w1t as [112, 7, 128] (rows 112-127 zero),
    #      cols 896..905 = w2t [128, 10]
    wpk = nc.dram_tensor("wpk", [HID, WCOL + OUT], wdt, kind="ExternalInput").ap()
    # bpk: col 0 = b1 [128], col 1 rows 0..9 = b2
    bpk = nc.dram_tensor("bpk", [HID, 2], f32, kind="ExternalInput").ap()
    outT = nc.dram_tensor("outT", [OUT, bc], f32, kind="ExternalOutput").ap()

    with ExitStack() as ctx:
        tc = ctx.enter_context(tile.TileContext(nc))
        wpool = ctx.enter_context(tc.tile_pool(name="w", bufs=1))
        xpool = ctx.enter_context(tc.tile_pool(name="x", bufs=len(blocks)))
        hpool = ctx.enter_context(tc.tile_pool(name="h", bufs=3))
        opool = ctx.enter_context(tc.tile_pool(name="o", bufs=3))
        p1pool = ctx.enter_context(tc.tile_pool(name="p1", bufs=2, space="PSUM"))
        p2pool = ctx.enter_context(tc.tile_pool(name="p2", bufs=2, space="PSUM"))

        ws = wpool.tile([HID, WCOL + OUT], wdt)
        bs = wpool.tile([HID, 2], f32)
        nc.scalar.dma_start(ws[:], wpk[:])
        nc.scalar.dma_start(bs[:], bpk[:])
        b1s = bs[:, 0:1]
        b2s = bs[0:OUT, 1:2]
        w2s = ws[:, WCOL : WCOL + OUT]

        xs_list = []
        off = 0
        for blk, xb in enumerate(blocks):
            xs = xpool.tile([KC, KCH * xb], xdt, tag="xs", name=f"xs_{blk}")
            nc.sync.dma_start(xs[:], xT[:, KCH * off : KCH * (off + xb)])
            xs_list.append(xs)
            off += xb

        add = mybir.AluOpType.add
        mx = mybir.AluOpType.max

        # units: (blk, xb, unit column offset in block, unit cols, global off)
        units = []
        off = 0
        for blk, xb in enumerate(blocks):
            for u0 in range(0, xb, UC):
                units.append((blk, xb, u0, min(UC, xb - u0), off + u0))
            off += xb

        def emit_l1(uidx):
            blk, xb, u0, uc, _ = units[uidx]
            xs = xs_list[blk]
            p1 = p1pool.tile([HID, uc], f32, tag="p1", name=f"p1_{uidx}")
            for t0 in range(0, uc, NT):
                nt = min(NT, uc - t0)
                for c in range(KCH):
                    col = c * xb + u0 + t0
                    nc.tensor.matmul(
                        p1[:, t0 : t0 + nt],
                        ws[0:KC, c * HID : (c + 1) * HID],
                        xs[:, col : col + nt],
                        start=(c == 0),
                        stop=(c == KCH - 1),
                    )
            return p1

        def emit_l2(uidx, p1):
            blk, xb, u0, uc, goff = units[uidx]
            hs = hpool.tile([HID, uc], xdt, tag="hs", name=f"hs_{uidx}")
            nc.vector.tensor_scalar(hs[:], p1[:], b1s, 0.0, add, mx)
            p2 = p2pool.tile([OUT, uc], f32, tag="p2", name=f"p2_{uidx}")
            for t0 in range(0, uc, NT):
                nt = min(NT, uc - t0)
                nc.tensor.matmul(
                    p2[:, t0 : t0 + nt], w2s, hs[:, t0 : t0 + nt],
                    start=True, stop=True,
                )
            os_ = opool.tile([OUT, uc], f32, tag="os", name=f"os_{uidx}")
            nc.vector.tensor_scalar_add(os_[:], p2[:], b2s)
            nc.scalar.dma_start(outT[:, goff : goff + uc], os_[:])

        prev = None
        for uidx in range(len(units)):
            p1 = emit_l1(uidx)
            if prev is not None:
                emit_l2(*prev)
            prev = (uidx, p1)
        emit_l2(*prev)

    nc.compile()
    return nc


def get_nc(bc=BC, variant=VARIANT):
    key = (bc, variant)
    if key not in _NC_CACHE:
        _NC_CACHE[key] = _build_nc(bc, variant)
    return _NC_CACHE[key]


def _np_wdt(variant):
    if variant == "bf16":
        import ml_dtypes

        return ml_dtypes.bfloat16
    if variant == "f16":
        return np.float16
    return np.float32


def _pack_xT(shard, blocks, wnp):
    """[bc, 784] row-major shard -> [112, 7*bc] per-block-contiguous."""
    parts = []
    off = 0
    for xb in blocks:
        sub = shard[off : off + xb]  # [xb, 784]
        # [xb, 7, 112] -> [112, 7, xb] -> [112, 7*xb]
        parts.append(sub.reshape(xb, KCH, KC).transpose(2, 1, 0).reshape(KC, KCH * xb))
        off += xb
    return np.ascontiguousarray(np.concatenate(parts, axis=1)).astype(wnp, copy=False)


def _host_prep(x, conv_w, w1, b1, w2, b2, variant):
    """Fold conv into layer-1 weights and lay out per-core device inputs."""
    x = np.asarray(x, dtype=np.float32)
    conv_w = np.asarray(conv_w, dtype=np.float32)
    w1 = np.asarray(w1, dtype=np.float32)
    b1 = np.asarray(b1, dtype=np.float32)
    w2 = np.asarray(w2, dtype=np.float32)
    b2 = np.asarray(b2, dtype=np.float32)

    w1_img = w1.reshape(HID, CH, CW)
    w1eff = np.zeros((HID, H, W), dtype=np.float32)
    for di in range(K):
        for dj in range(K):
            w1eff[:, di : di + CH, dj : dj + CW] += conv_w[di, dj] * w1_img
    w1eff = w1eff.reshape(HID, FEAT)

    wnp = _np_wdt(variant)
    # w1t layout [112, 7*128]: chunk c partition p holds feature c*112+p
    w1t_host = (
        w1eff.T.reshape(KCH, KC, HID).transpose(1, 0, 2).reshape(KC, KCH * HID)
    )
    wpk_host = np.zeros((HID, WCOL + OUT), dtype=np.float32)
    wpk_host[0:KC, 0:WCOL] = w1t_host
    wpk_host[:, WCOL : WCOL + OUT] = w2.T
    wpk_host = np.ascontiguousarray(wpk_host).astype(wnp)

    bpk_host = np.zeros((HID, 2), dtype=np.float32)
    bpk_host[:, 0] = b1
    bpk_host[0:OUT, 1] = b2
    bpk_host = np.ascontiguousarray(bpk_host)

    blocks = _blocks(BC)
    xq = x.astype(wnp)
    in_maps = []
    for c in range(NCORES):
        in_maps.append(
            {
                "xT": _pack_xT(xq[c * BC : (c + 1) * BC], blocks, wnp),
                "wpk": wpk_host,
                "bpk": bpk_host,
            }
        )
    return in_maps


def run(x, conv_w, w1, b1, w2, b2, trace=False, variant=VARIANT):
    from concourse.bass_utils import run_bass_kernel_spmd

    in_maps = _host_prep(x, conv_w, w1, b1, w2, b2, variant)
    nc = get_nc(BC, variant)
    res = run_bass_kernel_spmd(nc, in_maps, list(range(NCORES)), trace=trace)
    outT = np.concatenate([r["outT"] for r in res.results], axis=1)  # [10, B]
    return np.ascontiguousarray(outT.T), res


def kernel(x, conv_w, w1, b1, w2, b2):
    out, _ = run(x, conv_w, w1, b1, w2, b2)
    return out
